# revision 28
# baseline (speedup 1.0000x reference)
"""Trainium2 Bass kernel for nn_BaseRuleLearner (v3).

Math (per batch element b, reference semantics):
  UM[b,i,v,l]      = sum_e U[b,l,e]  * ru[i,v,e]
  BM[b,i,n,m,j,k]  = sum_e Bf[b,j,k,e] * rb[i,n,m,e]
  scores[b,i,p]    = sum_v UM[b,i,v,perm[p,v]]
                   + sum_{n,m} BM[b,i,n,m,perm[p,n],perm[p,m]]
  merged[b,i]      = min_p scores[b,i,p]
  out[b,:]         = softmax_i(merged) @ one_hot([0,0,1,1])

v3 design (pure data parallel over B across 8 cores, 512 b/core):

- Diagonal (n==m) binary terms are folded into the unary path on the
  host: unary contraction k=128 = [e_unary ; e_diag] with weights
  [ru[i,v,:] ; rb[i,v,v,:]].  Only the 6 off-diagonal (n,m) pairs are
  computed in the binary path -> 48 = 4i x 12t'' psum rows, t''=(s,od).
- Since permutations have distinct entries, off-diagonal (n,m) never
  gathers a diagonal jk — the 8 diagonal jk columns are dropped and the
  remaining 56 jk are re-paired into 28 k=128 blocks (w is pair-
  agnostic).  Input shrinks to 4.5 MB/core and stage-2 needs only
  3 k-chunks of 120 rows (48 matmuls total instead of 64).
- X [128, 18432] = [unary 8 l-blocks | 28 binary pair-blocks] x 512 b.
  Unary is placed FIRST so the unary matmuls attached to the first
  groups never stall the psum-buffer rotation.  All X DMAs issue on
  sync; scalar only issues G and stays free for evac.
- Stage 1: 14 groups (2 pairs x 512 b, 4 matmuls n=512 for groups 0-3
  which also run the unary quarters into psum rows 64:76).  One bf16
  evac op per psum region (ACT/DVE round-robin; Pool cannot touch
  PSUM on TRN2).
- sg staging [76, 28 x 528] bf16; assembly: 16 SBUF->SBUF DMAs
  (per-i, 3-dim APs, partition dim outermost, rows t-major) into
  qt chunk tiles; chunk readiness tracks evac order so assembly and
  early stage-2 overlap stage 1; the last chunk's 4 DMAs are split
  across engines for parallel descriptor-gen.
- Stage 2: per (i,bt): 3 bf16 matmuls (kc=120) accumulate
  psum[128b, 336p], order c1 -> c2 -> c0 (readiness order); bt0's
  partials are interleaved into the stage-1 PE stream; a second psum
  pool opened after stage-1's pool closes gives 8 units in flight.
- min over p on DVE; softmax over i=4 without max-shift (scores
  bounded, fp32 exp safe); pair-sum; one gathered output DMA.
"""

import itertools
import numpy as np

B, O, E = 4096, 8, 64
I, V = 4, 3
P = 336
N_CORES = 8
BC = B // N_CORES            # 512 batch per core
NPAIR = 28                   # re-paired off-diagonal jk blocks
NG = NPAIR // 2              # stage-1 groups (2 pairs each) = 14
BLK = 528                    # sg cols per block: 512 b + 16 pad
XB0 = O * BC                 # 4096: binary cols start (unary first)
XCOLS = XB0 + NPAIR * BC     # 18432
OD_IX = {1: 0, 2: 1, 3: 2, 5: 3, 6: 4, 7: 5}   # offdiag nm=(n*3+m) -> 0..5
# off-diagonal jk list and its pairing: pair q = (ODJK[2q], ODJK[2q+1])
ODJK = [jk for jk in range(O * O) if jk // O != jk % O]   # 56 entries
# chunks over pairs: c1 = pairs 0..9, c2 = 10..19, c0 = 20..27 + unary
# (c0 carries the last-arriving pairs AND the unary rows; accumulation
# order in stage 2 is c1, c2, c0)
KC = [120, 120, 120]         # rows per chunk (index = chunk id 0,1,2)
CHUNK_PAIRS = {1: (0, 10), 2: (10, 20), 0: (20, 28)}
NBT = BC // 128              # 4 b-tiles per core

_PERM = np.array(list(itertools.permutations(range(O), V)), dtype=np.int32)

_CACHED = {}


WAVES = {1: (0, 5, 10), 2: (10, 15, 20), 0: (20, 24, 28)}


def _chunk_waves(q):
    for c, (a, m, b) in WAVES.items():
        if a <= q < b:
            return c, (a, m, b), (24 if c == 0 else 0)
    raise AssertionError


def _pair_of_jk():
    m = {}
    for pos, jk in enumerate(ODJK):
        m[jk] = (pos // 2, pos % 2)
    return m


def _build_g():
    """G[c][r, p] 0/1 gather matrices.

    Chunk rows: each chunk is split into two WAVES (pair-subsets);
    rows are t-major WITHIN a wave so each wave is a legal
    partition-outermost DMA that fires as soon as its blocks are
    evac'd: r = wave_row0 + t''*wave_njp + (q - wave_a); c0 rows 0:24
    are unary (v*8 + l).
    """
    pm = _pair_of_jk()
    g = [np.zeros((KC[c], P), np.float32) for c in range(3)]
    for p in range(P):
        for n in range(V):
            for m in range(V):
                if n == m:
                    continue
                jk = int(_PERM[p, n]) * O + int(_PERM[p, m])
                q, s = pm[jk]
                od = OD_IX[n * V + m]
                t2 = s * 6 + od
                c, (a, m, b), r0 = _chunk_waves(q)
                if q < m:
                    g[c][r0 + t2 * (m - a) + (q - a), p] += 1.0
                else:
                    g[c][r0 + (m - a) * 12 + t2 * (b - m) + (q - m), p] += 1.0
        for v in range(V):
            l = int(_PERM[p, v])
            g[0][v * O + l, p] += 1.0
    return g


def _build_module():
    import concourse.tile as tile
    from concourse import bacc, mybir

    FP = mybir.dt.float32
    BF = mybir.dt.bfloat16
    AX = mybir.AxisListType.X
    nc = bacc.Bacc("TRN2", target_bir_lowering=False, debug=False)

    xd = nc.dram_tensor("x", [128, XCOLS], BF, kind="ExternalInput")
    wd = nc.dram_tensor("w", [128, 60], BF, kind="ExternalInput")
    gd = nc.dram_tensor("g", [128, 3 * P], BF, kind="ExternalInput")
    out = nc.dram_tensor("out", [BC, 4], FP, kind="ExternalOutput")

    with tile.TileContext(nc) as tc:
        with (
            tc.tile_pool(name="wpool", bufs=1) as wpool,
            tc.tile_pool(name="xpool", bufs=1) as xpool,
            tc.tile_pool(name="sgpool", bufs=1) as sgpool,
            tc.tile_pool(name="qpool", bufs=1) as qpool,
            tc.tile_pool(name="mpool", bufs=1) as mpool,
            tc.tile_pool(name="ps2", bufs=4, space="PSUM") as ps2,
            tc.tile_pool(name="ps1", bufs=4, space="PSUM") as ps1_pool,
        ):
            # ---- constants ----
            w_sb = wpool.tile([128, 60], BF, tag="w")
            nc.sync.dma_start(w_sb[:], wd.ap()[:])
            g_sb = wpool.tile([128, 3 * P], BF, tag="g")
            nc.scalar.dma_start(g_sb[:], gd.ap()[:])
            w_bin = w_sb[:, 0:48]
            w_un = w_sb[:, 48:60]

            # ---- input: 9 chunked DMAs, all on sync (scalar must stay
            # free for evac) ----
            x_sb = xpool.tile([128, XCOLS], BF, tag="x")
            cw = 2048
            spans = [(4096, 5120), (5120, 6144), (0, 2048), (2048, 4096)] + [
                (k * cw, (k + 1) * cw) for k in range(3, 9)
            ]
            for a, b in spans:
                nc.sync.dma_start(x_sb[:, a:b], xd.ap()[:, a:b])

            # ---- staging + chunk tiles ----
            sg = sgpool.tile([76, NPAIR * BLK], BF, tag="sg")
            sgv = sg[:].rearrange("p (j w) -> p j w", j=NPAIR)
            qt = [
                qpool.tile([KC[c], I * 512], BF, tag=f"q{c}", name=f"q{c}")
                for c in range(3)
            ]

            # PE p-state warmup: harmless matmuls on already-loaded
            # constants while the input DMAs are still in flight
            warm = ps2.tile([128, P], FP, tag="sc", name="warm")
            for _ in range(6):
                nc.tensor.matmul(
                    warm[0:48, :], w_bin, g_sb[:, 0:P], start=True, stop=True
                )

            merged = mpool.tile([128, 16], FP, tag="m")      # (bt, i)
            sums = mpool.tile([128, 4], FP, tag="s")
            ex = mpool.tile([128, 16], FP, tag="e")
            rc = mpool.tile([128, 4], FP, tag="r")
            fin = mpool.tile([128, 16], FP, tag="f")
            pr = mpool.tile([128, 16], FP, tag="p")

            # evac schedule: 36 per-block ops (ACT ~0.57us vs DVE ~0.66us)
            def ev_bin(q):
                return nc.scalar if (q % 2 == 0 or q == 27) else nc.vector

            def ev_un(q):
                return nc.scalar if q % 2 == 0 else nc.vector

            def evac(pb, q, eng, rows, part0):
                dst = sgv[part0 : part0 + rows, q, 0:512]
                src = pb[part0 : part0 + rows, 0:512]
                if eng is nc.scalar:
                    eng.copy(dst, src)
                else:
                    eng.tensor_copy(dst, src)

            # ---- assembly DMAs (per-i, partition dim outermost) ----
            def asm_binary(c, a2, b2, eng_list):
                a, m, b = WAVES[c]
                r0 = 24 if c == 0 else 0
                roff = r0 if a2 == a else r0 + (m - a) * 12
                for i in range(I):
                    srcv = (
                        sg[i * 12 : (i + 1) * 12, :]
                        .rearrange("t (j w) -> t j w", j=NPAIR)
                        [:, a2:b2, 0:512]
                    )
                    dst = qt[c][
                        roff : roff + (b2 - a2) * 12,
                        i * 512 : (i + 1) * 512,
                    ]
                    eng_list[i % len(eng_list)].dma_start(dst, srcv)

            def asm_unary(eng_list):
                for i in range(I):
                    srcu = (
                        sg[64 + i * 3 : 64 + (i + 1) * 3, :]
                        .rearrange("v (l w) -> v l w", l=NPAIR)
                        [:, 0:8, 0:512]
                    )
                    dstu = qt[0][0:24, i * 512 : (i + 1) * 512]
                    eng_list[i % len(eng_list)].dma_start(dstu, srcu)

            # ---- stage 1 ----
            # group g: pairs (2g, 2g+1) at X cols XB0 + g*1024;
            # g<4 also unary quarter g (X cols g*1024).
            def s1_group(g):
                for h in range(2):
                    q = 2 * g + h
                    pb = ps1_pool.tile([76, 512], FP, tag="pb")
                    nc.tensor.matmul(
                        pb[0:48, :],
                        w_bin,
                        x_sb[:, XB0 + q * 512 : XB0 + (q + 1) * 512],
                        start=True,
                        stop=True,
                    )
                    if g < 4:
                        nc.tensor.matmul(
                            pb[64:76, :],
                            w_un,
                            x_sb[:, q * 512 : (q + 1) * 512],
                            start=True,
                            stop=True,
                        )
                        # one [76,512] op: rows 48:63 are dead (never read)
                        evac(pb, q, ev_bin(q), 76, 0)
                    else:
                        evac(pb, q, ev_bin(q), 48, 0)

            # ---- stage 2 helpers ----
            S2_ORDER = [1, 2, 0]     # accumulation order (readiness)

            def s2_mm(sc, i, bt, c, start, stop):
                kc = KC[c]
                lhsT = (
                    qt[c][0:kc]
                    .rearrange("r (i b) -> r i b", i=I)
                    [:, i, bt * 128 : (bt + 1) * 128]
                )
                nc.tensor.matmul(
                    sc[:],
                    lhsT,
                    g_sb[0:kc, c * P : (c + 1) * P],
                    start=start,
                    stop=stop,
                )

            def s2_min(sc, i, bt):
                nc.vector.tensor_reduce(
                    merged[:, bt * 4 + i : bt * 4 + i + 1], sc[:], axis=AX,
                    op=mybir.AluOpType.min,
                )

            # ---- emission: stage 1, then assembly, then stage 2 ----
            UNITS = [(i, bt) for bt in range(NBT) for i in range(I)]
            nc.vector.memset(fin[:], 0.0)
            outv = out.ap().rearrange("(a p) m -> p a m", p=128)
            finv = fin[:].rearrange("p (a m) -> p a m", a=NBT)

            for g in range(3):
                s1_group(g)
            asm_binary(1, 0, 5, [nc.sync])      # pairs 0-4: after block 4
            s1_group(3)
            asm_unary([nc.gpsimd])              # unary: after blocks 0-7
            s1_group(4)
            asm_binary(1, 5, 10, [nc.sync])     # pairs 5-9: after block 9
            for g in range(5, 8):
                s1_group(g)
            asm_binary(2, 10, 15, [nc.gpsimd])  # pairs 10-14: after block 14
            s1_group(8)
            s1_group(9)
            asm_binary(2, 15, 20, [nc.sync])    # pairs 15-19: after block 19
            s1_group(10)
            s1_group(11)
            asm_binary(0, 20, 24, [nc.gpsimd])  # pairs 20-23: after block 23
            s1_group(12)
            s1_group(13)
            # pairs 24-27 arrive last: parallel issue on 3 engines
            asm_binary(0, 24, 28, [nc.sync, nc.scalar, nc.gpsimd, nc.sync])

            for u in range(16):
                i, bt = UNITS[u]
                sc = ps2.tile([128, P], FP, tag="sc", name=f"sc{u}")
                for ci, c in enumerate(S2_ORDER):
                    s2_mm(sc, i, bt, c, ci == 0, ci == 2)
                s2_min(sc, i, bt)
                if i == 3:
                    nc.scalar.activation(
                        ex[:, bt * 4 : bt * 4 + 4],
                        merged[:, bt * 4 : bt * 4 + 4],
                        mybir.ActivationFunctionType.Exp,
                        accum_out=sums[:, bt : bt + 1],
                    )
                    nc.vector.reciprocal(
                        rc[:, bt : bt + 1], sums[:, bt : bt + 1]
                    )
                    nc.vector.tensor_scalar_mul(
                        pr[:, bt * 4 : bt * 4 + 4],
                        ex[:, bt * 4 : bt * 4 + 4],
                        rc[:, bt : bt + 1],
                    )
                    prv = pr[:, bt * 4 : bt * 4 + 4].rearrange(
                        "p (a b) -> p a b", a=2
                    )
                    nc.vector.tensor_add(
                        fin[:, bt * 4 : bt * 4 + 2], prv[:, :, 0], prv[:, :, 1]
                    )
                    nc.sync.dma_start(outv[:, bt, :], finv[:, bt, :])

    nc.compile()
    return nc


def _get_module():
    if "nc" not in _CACHED:
        _CACHED["nc"] = _build_module()
    return _CACHED["nc"]


def _host_inputs(unary_feats, binary_feats, rule_unary, rule_binary):
    """Shard + lay out inputs for the 8 cores."""
    import ml_dtypes

    bf16 = ml_dtypes.bfloat16
    uf = np.asarray(unary_feats, dtype=np.float32).astype(bf16)
    bf = np.asarray(binary_feats, dtype=np.float32).astype(bf16)
    rbf = np.asarray(rule_binary, dtype=np.float32)
    ruf = np.asarray(rule_unary, dtype=np.float32)

    # w [128, 60]: binary cols (i, s*6+od) block-diag over s; unary cols
    # (i, v) with rows [ru ; rb_diag]
    w = np.zeros((128, 60), bf16)
    for i in range(I):
        for n in range(V):
            for m in range(V):
                if n == m:
                    continue
                od = OD_IX[n * V + m]
                w[0:64, i * 12 + 0 * 6 + od] = rbf[i, n, m].astype(bf16)
                w[64:128, i * 12 + 1 * 6 + od] = rbf[i, n, m].astype(bf16)
        for v in range(V):
            w[0:64, 48 + i * 3 + v] = ruf[i, v].astype(bf16)
            w[64:128, 48 + i * 3 + v] = rbf[i, v, v].astype(bf16)

    gs = _build_g()
    g = np.zeros((128, 3 * P), bf16)
    for c in range(3):
        g[0 : KC[c], c * P : (c + 1) * P] = gs[c].astype(bf16)

    in_maps = []
    for cidx in range(N_CORES):
        bfc = bf[cidx * BC : (cidx + 1) * BC]              # [BC, O, O, E]
        x = bfc.reshape(BC, O * O, E).transpose(1, 2, 0)   # [jk, e, b]
        xod = x[ODJK]                                      # [56, e, b]
        ab = np.ascontiguousarray(
            xod.reshape(NPAIR, 2, E, BC).transpose(1, 2, 0, 3)
        ).reshape(128, NPAIR * BC)                         # [(s,e), (q,b)]
        ufc = uf[cidx * BC : (cidx + 1) * BC]              # [BC, O, E]
        xu = ufc.transpose(1, 2, 0)                        # [l, e, b]
        diag = bfc[:, np.arange(O), np.arange(O), :]       # [BC, O, E]
        xdg = diag.transpose(1, 2, 0)                      # [l, e2, b]
        au2 = np.ascontiguousarray(
            np.concatenate([xu, xdg], axis=1).transpose(1, 0, 2)
        ).reshape(128, O * BC)                             # [(e,e2), (l,b)]
        X = np.ascontiguousarray(np.concatenate([au2, ab], axis=1))
        in_maps.append({"x": X, "w": w, "g": g})
    return in_maps


TRACE = False  # set True (e.g. from test.py) to capture an NTFF profile


def kernel(unary_feats, binary_feats, rule_unary, rule_binary):
    from concourse.bass_utils import run_bass_kernel_spmd

    nc = _get_module()
    in_maps = _host_inputs(unary_feats, binary_feats, rule_unary, rule_binary)
    res = run_bass_kernel_spmd(
        nc, in_maps, core_ids=list(range(N_CORES)), trace=TRACE
    )
    _CACHED["last_results"] = res
    return np.concatenate(
        [res.results[c]["out"] for c in range(N_CORES)], axis=0
    )
